# revision 1
# baseline (speedup 1.0000x reference)
"""Trainium2 Bass kernel for nn_Atten2Map (DeePMD dpa2 Atten2Map-style sparse attention).

Contract: kernel(**inputs) takes FULL unsharded numpy inputs
(g2 [2,512,128,64], h2 [2,512,128,3], nlist_mask [2,512,128] bool,
sw [2,512,128], Wqk [64,512]) and returns the full output
[2,512,128,128,4] float32. Internally shards the nb*nloc=1024 atoms
data-parallel across 8 NeuronCores.

Math per atom (nnei=128 neighbors, ND=64, NH=4 heads):
  qk   = g2 @ Wqk                  -> q_h, k_h     [128, 64] each
  raw  = q_h @ k_h^T / sqrt(64)    (scores)
  hh   = h2 @ h2^T                 (gate)
  t    = (raw * hh + 20) * sw_i * sw_j - 20
  a    = softmax(t, axis=-1)  (the -20 shift cancels in softmax)
  out[i, j, h] = a * mask_i * mask_j * sw_i * sw_j * hh / sqrt(3)

Device formulation (fp16 matmul operands = 10-bit mantissa, fp32 accumulate;
numerically equivalent to TF32, validated at relL2 ~7.6e-4):
  W2_h   = Wq_h @ Wk_h^T / 8       (host, 64x64; scores = G @ W2_h @ G^T)
  G^T    via DMA transpose (fp16, 2-byte XBAR path)
  tmpT_h = W2_h^T @ G^T            (PE)  [64(e), 128(i) x atom-pair]
  X_h    = tmpT_h^T @ G^T          (PE)  scores/8
  hhsw   = h2 @ (h2*sw)^T          (PE)  folds hh*sw_j
  hhm    = h2 @ (h2*mask*sw)^T     (PE)  folds hh*mask_j*sw_j
  V1     = (X * sw_i) * hhsw       (DVE scalar_tensor_tensor, PSUM read)
  V2     = V1 + (20*sw_i)*sw_j     (GPSIMD tensor_tensor; w20 via DVE 2x)
  E_h, rowsum_h = exp(V2_h - 60)   (ACT, fused accumulate)
  rinv'  = (1/rowsum)*mask_i*sw_i/sqrt(3)  (DVE, [128,4])
  out_h  = (E_h * rinv'_h) * hhm   (DVE STT, strided write -> [i, j*4+h])
"""

import numpy as np
from contextlib import ExitStack

import concourse.bass as bass
import concourse.tile as tile
from concourse import bacc, mybir
from concourse.bass_utils import run_bass_kernel_spmd

ND, NH, SHIFT = 64, 4, 20.0
NNEI, DIN = 128, 64
NCORES = 8
EXPB = 60.0  # constant shift inside exp; cancels in softmax normalization

F32 = mybir.dt.float32
F16 = mybir.dt.float16

P = NNEI  # 128


def _r3(ap):
    """[128, n*128] AP viewed as [128, n, 128]."""
    n = ap.shape[1] // P
    return ap.rearrange("p (h j) -> p h j", h=n)


def build_nc(A: int):
    """Build the per-core Bass program for A atoms (A even)."""
    assert A % 2 == 0
    nc = bacc.Bacc("TRN2", target_bir_lowering=False, debug=False, num_devices=NCORES)
    dp = nc.declare_dram_parameter
    g2T = dp("g2T", [A, DIN, P], F16, isOutput=False)
    h2T = dp("h2T", [A, 3, P], F16, isOutput=False)
    h2swT = dp("h2swT", [A, 3, P], F16, isOutput=False)
    h2mT = dp("h2mT", [A, 3, P], F16, isOutput=False)
    w2p = dp("w2p", [DIN, NH * ND], F16, isOutput=False)
    sws = dp("sws", [P, 3 * A], F32, isOutput=False)       # [swiT | swi20T | rmT]
    swrow = dp("swrow", [1, A * P], F32, isOutput=False)
    out = dp("out", [A, P, P * NH], F32, isOutput=True)

    AF = mybir.ActivationFunctionType
    OP = mybir.AluOpType

    with tile.TileContext(nc) as tc, ExitStack() as ctx:
        sb = ctx.enter_context(tc.tile_pool(name="persist", bufs=1))
        w2p_s = sb.tile([DIN, NH * ND], F16)
        nc.gpsimd.dma_start(w2p_s[:, :], w2p[:, :])
        sws_s = sb.tile([P, 3 * A], F32)
        nc.gpsimd.dma_start(sws_s[:, :], sws[:, :])
        swiT_s = sws_s[:, 0:A]
        swi20T_s = sws_s[:, A:2 * A]
        rmT_s = sws_s[:, 2 * A:3 * A]
        negb = sb.tile([P, 1], F32)
        nc.vector.memset(negb[:, :], -EXPB)

        # pools
        ht_pool = ctx.enter_context(tc.tile_pool(name="ht", bufs=3))
        gt_pool = ctx.enter_context(tc.tile_pool(name="gt", bufs=4))
        tts_pool = ctx.enter_context(tc.tile_pool(name="tts", bufs=2))
        hh_pool = ctx.enter_context(tc.tile_pool(name="hh", bufs=6))
        work_pool = ctx.enter_context(tc.tile_pool(name="work", bufs=3))
        stat_pool = ctx.enter_context(tc.tile_pool(name="stat", bufs=6))
        swj_pool = ctx.enter_context(tc.tile_pool(name="swj", bufs=2))
        # PSUM pools
        ptm_pool = ctx.enter_context(tc.tile_pool(name="ptm", bufs=1, space="PSUM"))
        psc_pool = ctx.enter_context(tc.tile_pool(name="psc", bufs=2, space="PSUM"))
        pmisc_pool = ctx.enter_context(tc.tile_pool(name="pmisc", bufs=1, space="PSUM"))

        for p in range(A // 2):
            a0, a1 = 2 * p, 2 * p + 1
            # --- H^T tiles: atoms stacked at partition rows {0:3, 64:67}
            ht = ht_pool.tile([3, 2 * P], F16, tag="ht")
            nc.gpsimd.dma_start(ht[0:3, 0:P], h2T[a0, :, :])
            nc.gpsimd.dma_start(ht[0:3, P:], h2T[a1, :, :])
            htsw = ht_pool.tile([3, 2 * P], F16, tag="htsw")
            nc.gpsimd.dma_start(htsw[0:3, 0:P], h2swT[a0, :, :])
            nc.gpsimd.dma_start(htsw[0:3, P:], h2swT[a1, :, :])
            htm = ht_pool.tile([3, 2 * P], F16, tag="htm")
            nc.gpsimd.dma_start(htm[0:3, 0:P], h2mT[a0, :, :])
            nc.gpsimd.dma_start(htm[0:3, P:], h2mT[a1, :, :])

            # --- G^T (host-pre-transposed, contiguous): [64, 256], atoms side by side
            gts = gt_pool.tile([DIN, 2 * P], F16)
            nc.gpsimd.dma_start(gts[:, 0:P], g2T[a0, :, :])
            nc.gpsimd.dma_start(gts[:, P:], g2T[a1, :, :])

            # --- tmpT matmuls: per head [64, 256] at base partition 0 -> SBUF [64, 1024]
            tts = tts_pool.tile([DIN, NH * 2 * P], F16)
            for hp in range(2):
                ptm = ptm_pool.tile([DIN, 4 * P], F32)
                for hi in range(2):
                    h = 2 * hp + hi
                    nc.tensor.matmul(ptm[:, hi * 2 * P:(hi + 1) * 2 * P],
                                     w2p_s[:, h * ND:(h + 1) * ND], gts[:, :],
                                     start=True, stop=True)
                nc.scalar.copy(tts[:, hp * 4 * P:(hp + 1) * 4 * P], ptm[:, :])

            # --- hhsw / hhm matmuls (even rows 0:3, odd rows 64:67)
            # --- hhsw / hhm pair matmuls (half the columns are cross-atom garbage)
            phh = pmisc_pool.tile([P, 4 * P], F32, tag="pmisc")
            nc.tensor.matmul(phh[:, 0:2 * P], ht[:, 0:P], htsw[:, :], start=True, stop=True)
            nc.tensor.matmul(phh[:, 2 * P:], ht[:, P:], htsw[:, :], start=True, stop=True)
            phm = pmisc_pool.tile([P, 4 * P], F32, tag="pmisc")
            nc.tensor.matmul(phm[:, 0:2 * P], ht[:, 0:P], htm[:, :], start=True, stop=True)
            nc.tensor.matmul(phm[:, 2 * P:], ht[:, P:], htm[:, :], start=True, stop=True)
            # merged copies: useful quarters [0:128] and [384:512] in one strided op
            hhs = hh_pool.tile([P, 2 * P], F32, tag="hh")
            nc.scalar.copy(hhs[:, :].rearrange("p (a j) -> p a j", a=2),
                           phh[:, :].rearrange("p (a j) -> p a j", a=4)[:, 0::3, :])
            hms = hh_pool.tile([P, 2 * P], F32, tag="hm")
            nc.scalar.copy(hms[:, :].rearrange("p (a j) -> p a j", a=2),
                           phm[:, :].rearrange("p (a j) -> p a j", a=4)[:, 0::3, :])
            # --- sw_j broadcast rows (exact fp32): DMA from DRAM, partition-broadcast source
            swjb = swj_pool.tile([P, 2 * P], F32)
            nc.gpsimd.dma_start(swjb[:, :],
                              swrow[0:1, a0 * P:(a0 + 2) * P].broadcast_to([P, 2 * P]))

            for ai, a in ((0, a0), (1, a1)):
                # --- scores: 2 head-pair PSUM tiles [128, 512] each (N=256, half garbage)
                v1 = work_pool.tile([P, 4 * P], F32, tag="v1")
                for hp in range(2):
                    psc = psc_pool.tile([P, 4 * P], F32)
                    for hi in range(2):
                        h = 2 * hp + hi
                        c0 = h * 2 * P + ai * P
                        nc.tensor.matmul(psc[:, hi * 2 * P:(hi + 1) * 2 * P],
                                         tts[:, c0:c0 + P], gts[:, :],
                                         start=True, stop=True)
                    x_ap = psc[:, :].rearrange("p (h j) -> p h j", h=2)[:, :, ai * P:(ai + 1) * P]
                    hh_b = hhs[:, ai * P:(ai + 1) * P].unsqueeze(1).broadcast_to([P, 2, P])
                    nc.vector.scalar_tensor_tensor(
                        _r3(v1[:, hp * 2 * P:(hp + 1) * 2 * P]),
                        x_ap, swiT_s[:, a:a + 1], hh_b,
                        op0=OP.mult, op1=OP.mult)
                # --- V2 = V1 + (20*sw_i)*sw_j
                w20 = stat_pool.tile([P, P], F32, tag="w20")
                nc.vector.tensor_scalar(
                    w20[:, :], swjb[:, ai * P:(ai + 1) * P], swi20T_s[:, a:a + 1], None,
                    op0=OP.mult)
                v2 = work_pool.tile([P, 4 * P], F32, tag="v2")
                w20_b = w20[:, :].unsqueeze(1).broadcast_to([P, NH, P])
                nc.gpsimd.tensor_tensor(
                    _r3(v2[:, :]), _r3(v1[:, :]), w20_b, op=OP.add)
                # --- E = exp(V2 - 60), fused row sums
                e_t = work_pool.tile([P, 4 * P], F32, tag="e")
                rows = stat_pool.tile([P, 3 * NH], F32, tag="rows")
                for h in range(NH):
                    nc.scalar.activation(
                        e_t[:, h * P:(h + 1) * P], v2[:, h * P:(h + 1) * P],
                        AF.Exp, bias=negb[:, 0:1], scale=1.0,
                        accum_out=rows[:, h:h + 1])
                nc.vector.reciprocal(rows[:, NH:2 * NH], rows[:, 0:NH])
                nc.vector.tensor_scalar(
                    rows[:, 2 * NH:], rows[:, NH:2 * NH], rmT_s[:, a:a + 1], None,
                    op0=OP.mult)
                # --- out_h = (E_h * rinv'_h) * hhm, interleaved write [i, j*4+h]
                ti = work_pool.tile([P, 4 * P], F32, tag="ti")
                ti3 = ti[:, :].rearrange("p (j h) -> p j h", h=NH)
                for h in range(NH):
                    nc.vector.scalar_tensor_tensor(
                        ti3[:, :, h], e_t[:, h * P:(h + 1) * P],
                        rows[:, 2 * NH + h:2 * NH + h + 1], hms[:, ai * P:(ai + 1) * P],
                        op0=OP.mult, op1=OP.mult)
                nc.gpsimd.dma_start(out[a, :, :], ti[:, :])
    if not nc.is_finalized():
        nc.finalize()
    return nc


def _host_prep(g2, h2, nlist_mask, sw, Wqk):
    """Build per-core input maps (host-side numpy prep)."""
    nb, nloc, nnei, din = g2.shape
    ATOT = nb * nloc
    A = ATOT // NCORES
    g2Tf = np.ascontiguousarray(g2.reshape(ATOT, nnei, din).transpose(0, 2, 1)).astype(np.float16)
    h2f = h2.reshape(ATOT, nnei, 3).astype(np.float32)
    maskf = nlist_mask.reshape(ATOT, nnei)
    swf = sw.reshape(ATOT, nnei).astype(np.float32)

    msw = swf * maskf  # [ATOT, 128]
    h2Tf = np.ascontiguousarray(h2f.transpose(0, 2, 1)).astype(np.float16)
    h2swTf = np.ascontiguousarray((h2f * swf[:, :, None]).transpose(0, 2, 1)).astype(np.float16)
    h2mTf = np.ascontiguousarray((h2f * msw[:, :, None]).transpose(0, 2, 1)).astype(np.float16)

    # W2 per head: Wqk columns c = d*8 + h; q heads h<4, k heads h>=4
    Wqk64 = Wqk.astype(np.float64).reshape(din, ND, 2 * NH)
    w2p = np.zeros((din, NH * ND), np.float16)
    for h in range(NH):
        Wq = Wqk64[:, :, h]          # [64, 64]
        Wk = Wqk64[:, :, NH + h]
        W2 = (Wq @ Wk.T) / np.sqrt(np.float64(ND))
        w2p[:, h * ND:(h + 1) * ND] = W2.astype(np.float16)

    in_maps = []
    for c in range(NCORES):
        s = slice(c * A, (c + 1) * A)
        sws = np.concatenate([swf[s].T, (SHIFT * swf[s]).T,
                              (msw[s] / np.sqrt(np.float32(3.0))).T], axis=1)
        in_maps.append({
            "g2T": g2Tf[s],
            "h2T": h2Tf[s],
            "h2swT": h2swTf[s],
            "h2mT": h2mTf[s],
            "w2p": w2p,
            "sws": np.ascontiguousarray(sws),
            "swrow": np.ascontiguousarray(swf[s].reshape(1, A * P)),
        })
    return in_maps, A


_NC_CACHE = {}


def kernel(g2, h2, nlist_mask, sw, Wqk, _trace=False, _trace_kwargs=None):
    nb, nloc, nnei, din = g2.shape
    in_maps, A = _host_prep(g2, h2, nlist_mask, sw, Wqk)
    key = A
    if key not in _NC_CACHE:
        _NC_CACHE[key] = build_nc(A)
    nc = _NC_CACHE[key]
    kw = {}
    if _trace:
        kw = dict(trace=True, **(_trace_kwargs or {}))
    res = run_bass_kernel_spmd(nc, in_maps, list(range(NCORES)), **kw)
    outs = [res.results[c]["out"] for c in range(NCORES)]
    full = np.concatenate(outs, axis=0)  # [1024, 128, 512]
    out = full.reshape(nb, nloc, nnei, nnei, NH).astype(np.float32)
    if _trace:
        return out, res
    return out


if __name__ == "__main__":
    import reference as R
    inputs = {k: np.asarray(v) for k, v in R.setup_inputs().items()}
    out = kernel(**inputs)
    import jax.numpy as jnp
    ref = np.asarray(R.reference(**{k: jnp.asarray(v) for k, v in inputs.items()}))
    err = np.abs(out - ref)
    scale = np.abs(ref).max()
    print("absmax err:", err.max(), "scale:", scale, "scale-rel:", err.max() / scale)
    print("rel L2:", np.linalg.norm(err) / np.linalg.norm(ref))



# revision 24
# speedup vs baseline: 1.3506x; 1.3506x over previous
"""Trainium2 Bass kernel for nn_Atten2Map (DeePMD dpa2 Atten2Map-style sparse attention).

Contract: kernel(**inputs) takes FULL unsharded numpy inputs
(g2 [2,512,128,64], h2 [2,512,128,3], nlist_mask [2,512,128] bool,
sw [2,512,128], Wqk [64,512]) and returns the full output
[2,512,128,128,4] float32. Internally shards the nb*nloc=1024 atoms
data-parallel across 8 NeuronCores.

Math per atom (nnei=128 neighbors, ND=64, NH=4 heads):
  raw_h = G W2_h G^T / 8        (W2_h = Wq_h Wk_h^T, host-folded)
  hh    = h2 h2^T
  t     = (raw*hh + 20) * sw_i * sw_j - 20
  a     = softmax(t, axis=-1)
  out[i,j,h] = a * mask_i * mask_j * sw_i * sw_j * hh / sqrt(3)

Device dataflow per atom pair (all engines balanced, SP issues DMAs on
the hardware DGE queue so GPSIMD is free for compute):
  PE:  ptm = [W2_01|W2_23]^T G^T  (2 mm, N=256)     stage-1
       phh = ht_a^T [hsw_a|hm_a]  (2 mm, N=256)     hh*sw_j and hh*mask*sw_j
       pw20 = ones^T [20sw_hi;20sw_lo] (1 mm, N=256) 20*sw_j bcast rows
       psc_a = tts_h^T gts_a      (8 mm, N=128)     scores
  ACT: tts = fp16(ptm); E_a = exp(swi * v2_a - 60)  (scale=per-row AP)
  GPSIMD: hhs = fp16(phh); w20s = f32(pw20); t_a1 = psc_a1 (*) hhsw
  DVE: t_a0 = psc_a0 (*) hhsw; v2 = t + w20; rowsums; recip; final
       out_a[i,(h,j)] = E * rinv_m[h] * hhm   (fp16, 4x mode)
Host: fp32 convert + (h,j)->(j,h) transpose of the output.
"""

import numpy as np
from contextlib import ExitStack

import concourse.bass as bass
import concourse.tile as tile
from concourse import bacc, mybir
from concourse.bass_utils import run_bass_kernel_spmd

ND, NH, SHIFT = 64, 4, 20.0
NNEI, DIN = 128, 64
NCORES = 8
EXPB = 60.0  # constant shift inside exp; cancels in softmax normalization

F32 = mybir.dt.float32
F16 = mybir.dt.float16
BF16 = mybir.dt.bfloat16

P = NNEI  # 128


def _r3(ap, n=NH):
    """[128, n*128] AP viewed as [128, n, 128]."""
    return ap.rearrange("p (h j) -> p h j", h=n)


def build_nc(A: int):
    """Per-core Bass program for A atoms (A even)."""
    assert A % 2 == 0
    NPAIR = A // 2
    nc = bacc.Bacc("TRN2", target_bir_lowering=False, debug=False, num_devices=NCORES)
    dp = nc.declare_dram_parameter
    g2Tp = dp("g2Tp", [NPAIR, DIN, 2 * P], F16, isOutput=False)
    htp = dp("htp", [NPAIR, 6, 2 * P], F16, isOutput=False)  # [h2T; h2T*rm]
    hhrhs = dp("hhrhs", [NPAIR, 3, 4 * P], F16, isOutput=False)
    w20rhs = dp("w20rhs", [NPAIR, 2, 2 * P], F16, isOutput=False)
    w2p = dp("w2p", [DIN, NH * DIN], F16, isOutput=False)
    sws = dp("sws", [P, A], F32, isOutput=False)  # swiT
    out = dp("out", [A, P, NH * P], F16, isOutput=True)

    AF = mybir.ActivationFunctionType
    OP = mybir.AluOpType

    with tile.TileContext(nc) as tc, ExitStack() as ctx:
        sb = ctx.enter_context(tc.tile_pool(name="persist", bufs=1))
        w2p_s = sb.tile([DIN, NH * DIN], F16)
        nc.gpsimd.dma_start(w2p_s[:, :], w2p[:, :])
        sws_s = sb.tile([P, A], F32)
        nc.gpsimd.dma_start(sws_s[:, :], sws[:, :])
        swiT_s = sws_s[:, 0:A]
        ones2 = sb.tile([2, P], F16)
        nc.vector.memset(ones2[:, :], 1.0)
        negb = sb.tile([P, 1], F32)
        nc.vector.memset(negb[:, :], -EXPB)

        gt_pool = ctx.enter_context(tc.tile_pool(name="gt", bufs=3))
        ht_pool = ctx.enter_context(tc.tile_pool(name="ht", bufs=3))
        tts_pool = ctx.enter_context(tc.tile_pool(name="tts", bufs=2))
        hh_pool = ctx.enter_context(tc.tile_pool(name="hh", bufs=2))
        work_pool = ctx.enter_context(tc.tile_pool(name="work", bufs=3))
        e_pool = ctx.enter_context(tc.tile_pool(name="e", bufs=3))
        stat_pool = ctx.enter_context(tc.tile_pool(name="stat", bufs=4))
        ti_pool = ctx.enter_context(tc.tile_pool(name="ti", bufs=3))
        # PSUM pools (8 banks total; 1+2+1+3 = 7 here)
        ptm_pool = ctx.enter_context(tc.tile_pool(name="ptm", bufs=1, space="PSUM"))
        phh_pool = ctx.enter_context(tc.tile_pool(name="phh", bufs=2, space="PSUM"))
        pw20_pool = ctx.enter_context(tc.tile_pool(name="pw20", bufs=1, space="PSUM"))
        psc_pool = ctx.enter_context(tc.tile_pool(name="psc", bufs=3, space="PSUM"))

        for p in range(NPAIR):
            a0 = 2 * p
            # --- input loads (SP hardware-DGE queue)
            # G^T replicated into both partition halves so odd-head matmuls
            # (lhsT at base partition 64) see a base-aligned rhs.
            gts = gt_pool.tile([DIN, 2 * P], F16)
            nc.gpsimd.dma_start(gts[:, :], g2Tp[p, :, :])
            ht = ht_pool.tile([3, 2 * P], F16, tag="ht")
            nc.gpsimd.dma_start(ht[:, :], htp[p, 0:3, :])
            htm = ht_pool.tile([3, 2 * P], F16, tag="htm")
            nc.gpsimd.dma_start(htm[:, :], htp[p, 3:6, :])
            hhr = ht_pool.tile([3, 4 * P], F16, tag="hhr")
            nc.gpsimd.dma_start(hhr[:, :], hhrhs[p, :, :])
            w20r = ht_pool.tile([2, 2 * P], F16, tag="w20r")
            nc.gpsimd.dma_start(w20r[:, :], w20rhs[p, :, :])

            # --- stage-1: tmpT per head, heads along columns (base partition 0)
            ptm = ptm_pool.tile([DIN, 8 * P], F32)
            for h in range(NH):
                nc.tensor.matmul(ptm[:, h * 2 * P:(h + 1) * 2 * P],
                                 w2p_s[:, h * DIN:(h + 1) * DIN], gts[:, :],
                                 start=True, stop=True)
            tts = tts_pool.tile([DIN, 8 * P], F16)
            nc.scalar.copy(tts[:, :], ptm[:, :])

            # --- hh gates: per atom [hhsw | hhm], plus 20*sw_j rows
            phh = phh_pool.tile([P, 4 * P], F32)
            for ai in range(2):
                c0 = ai * 2 * P
                nc.tensor.matmul(phh[:, c0:c0 + P], ht[:, ai * P:(ai + 1) * P],
                                 hhr[:, c0:c0 + P], start=True, stop=True)
                nc.tensor.matmul(phh[:, c0 + P:c0 + 2 * P],
                                 htm[:, ai * P:(ai + 1) * P],
                                 hhr[:, c0 + P:c0 + 2 * P], start=True, stop=True)
            pw20 = pw20_pool.tile([P, 2 * P], F32)
            nc.tensor.matmul(pw20[:, :], ones2[:, :], w20r[:, :],
                             start=True, stop=True)
            hhs = hh_pool.tile([P, 4 * P], F16, tag="hhs")
            nc.scalar.copy(hhs[:, :], phh[:, :])
            w20s = hh_pool.tile([P, 2 * P], F32, tag="w20s")
            nc.scalar.copy(w20s[:, :], pw20[:, :])

            for ai in range(2):
                a = a0 + ai
                # --- scores for atom a, 4 heads into one PSUM bank
                psc = psc_pool.tile([P, 4 * P], F32)
                for h in range(NH):
                    nc.tensor.matmul(
                        psc[:, h * P:(h + 1) * P],
                        tts[:, h * 2 * P + ai * P:h * 2 * P + ai * P + P],
                        gts[:, ai * P:(ai + 1) * P],
                        start=True, stop=True)
                # --- t = psc * hhsw_a  (DVE; PSUM read forbidden on GPSIMD)
                hhsw_b = hhs[:, ai * 2 * P:ai * 2 * P + P].unsqueeze(1) \
                    .broadcast_to([P, NH, P])
                t = work_pool.tile([P, 4 * P], F32, tag="t")
                nc.vector.tensor_tensor(_r3(t[:, :]), _r3(psc[:, :]), hhsw_b,
                                        op=OP.mult)
                # --- v2 = t + 20*sw_j  (SBUF only: DVE for a0, GPSIMD for a1)
                w20_b = w20s[:, ai * P:(ai + 1) * P].unsqueeze(1) \
                    .broadcast_to([P, NH, P])
                v2 = work_pool.tile([P, 4 * P], F32, tag="v2")
                nc.gpsimd.tensor_tensor(_r3(v2[:, :]), _r3(t[:, :]), w20_b,
                                        op=OP.add)
                # --- E = exp(swi*v2 - 60)
                e_t = e_pool.tile([P, 4 * P], F32)
                nc.scalar.activation(e_t[:, :], v2[:, :], AF.Exp,
                                     bias=negb[:, 0:1], scale=swiT_s[:, a:a + 1])
                # --- row sums per head, rinv = 1 / rowsum
                rows = stat_pool.tile([P, 2 * NH], F32)
                nc.vector.tensor_reduce(rows[:, 0:NH], _r3(e_t[:, :]),
                                        axis=mybir.AxisListType.X, op=OP.add)
                nc.vector.reciprocal(rows[:, NH:2 * NH], rows[:, 0:NH])
                # --- out[i,(h,j)] = E * rinv_h * hhm'_a  (GPSIMD, SBUF only;
                #     rm = mask_i*sw_i/sqrt(3) is folded into hhm' via htm)
                ti = ti_pool.tile([P, 4 * P], F16)
                hhm_a = hhs[:, ai * 2 * P + P:(ai + 1) * 2 * P]
                for h in range(NH):
                    nc.vector.scalar_tensor_tensor(
                        ti[:, h * P:(h + 1) * P], e_t[:, h * P:(h + 1) * P],
                        rows[:, NH + h:NH + h + 1], hhm_a,
                        op0=OP.mult, op1=OP.mult)
                nc.gpsimd.dma_start(out[a, :, :], ti[:, :])
    if not nc.is_finalized():
        nc.finalize()
    return nc


def _host_prep(g2, h2, nlist_mask, sw, Wqk):
    """Build per-core input maps (host-side numpy prep)."""
    nb, nloc, nnei, din = g2.shape
    ATOT = nb * nloc
    A = ATOT // NCORES
    NPAIR = A // 2
    g2f = np.asarray(g2, np.float32).reshape(ATOT, nnei, din)
    h2f = np.asarray(h2, np.float32).reshape(ATOT, nnei, 3)
    maskf = np.asarray(nlist_mask).reshape(ATOT, nnei)
    swf = np.asarray(sw, np.float32).reshape(ATOT, nnei)

    # pair-packed G^T: [pair, 64, (a0 j | a1 j)]
    g2T = np.ascontiguousarray(g2f.transpose(0, 2, 1)).astype(np.float16)
    g2Tp = np.ascontiguousarray(
        g2T.reshape(ATOT // 2, 2, din, nnei).transpose(0, 2, 1, 3)
    ).reshape(ATOT // 2, din, 2 * nnei)
    # pair-packed h2^T: rows 0-2 plain, rows 3-5 scaled by rm = mask*sw/sqrt(3)
    g2T = None  # free
    msw = swf * maskf
    rm = (msw / np.sqrt(np.float32(3.0)))
    h2Tf = h2f.transpose(0, 2, 1).astype(np.float16)  # [ATOT, 3, 128]
    h2Tm = (h2f * rm[:, :, None]).transpose(0, 2, 1).astype(np.float16)
    htp = np.empty((ATOT // 2, 6, 2 * nnei), np.float16)
    htp[:, 0:3, :] = h2Tf.reshape(ATOT // 2, 2, 3, nnei) \
        .transpose(0, 2, 1, 3).reshape(ATOT // 2, 3, 2 * nnei)
    htp[:, 3:6, :] = h2Tm.reshape(ATOT // 2, 2, 3, nnei) \
        .transpose(0, 2, 1, 3).reshape(ATOT // 2, 3, 2 * nnei)
    # hh rhs: [pair, 3, (hsw_a0 | hm_a0 | hsw_a1 | hm_a1)]
    hsw = (h2f * swf[:, :, None]).transpose(0, 2, 1).astype(np.float16)
    hm = (h2f * msw[:, :, None]).transpose(0, 2, 1).astype(np.float16)
    hhrhs = np.empty((ATOT // 2, 3, 4 * nnei), np.float16)
    hswp = hsw.reshape(ATOT // 2, 2, 3, nnei)
    hmp = hm.reshape(ATOT // 2, 2, 3, nnei)
    hhrhs[:, :, 0 * nnei:1 * nnei] = hswp[:, 0]
    hhrhs[:, :, 1 * nnei:2 * nnei] = hmp[:, 0]
    hhrhs[:, :, 2 * nnei:3 * nnei] = hswp[:, 1]
    hhrhs[:, :, 3 * nnei:4 * nnei] = hmp[:, 1]
    # 20*sw_j in exact hi/lo fp16 split: [pair, 2, (a0 | a1)]
    v20 = SHIFT * swf
    hi = v20.astype(np.float16)
    lo = (v20 - hi.astype(np.float32)).astype(np.float16)
    w20rhs = np.empty((ATOT // 2, 2, 2 * nnei), np.float16)
    w20rhs[:, 0, :] = hi.reshape(ATOT // 2, 2 * nnei)
    w20rhs[:, 1, :] = lo.reshape(ATOT // 2, 2 * nnei)

    # W2 per head: Wqk columns c = d*8 + h; q heads h<4, k heads h>=4
    Wqk64 = np.asarray(Wqk, np.float64).reshape(din, ND, 2 * NH)
    w2p = np.zeros((din, NH * ND), np.float16)
    for h in range(NH):
        Wq = Wqk64[:, :, h]
        Wk = Wqk64[:, :, NH + h]
        w2p[:, h * ND:(h + 1) * ND] = ((Wq @ Wk.T) / np.sqrt(np.float64(ND))
                                       ).astype(np.float16)

    in_maps = []
    for c in range(NCORES):
        s = slice(c * A, (c + 1) * A)
        sp = slice(c * NPAIR, (c + 1) * NPAIR)
        swsc = swf[s].T
        in_maps.append({
            "g2Tp": g2Tp[sp],
            "htp": htp[sp],
            "hhrhs": hhrhs[sp],
            "w20rhs": w20rhs[sp],
            "w2p": w2p,
            "sws": np.ascontiguousarray(swsc),
        })
    return in_maps, A


_NC_CACHE = {}


def kernel(g2, h2, nlist_mask, sw, Wqk, _trace=False, _trace_kwargs=None):
    nb, nloc, nnei, din = g2.shape
    in_maps, A = _host_prep(g2, h2, nlist_mask, sw, Wqk)
    key = A
    if key not in _NC_CACHE:
        _NC_CACHE[key] = build_nc(A)
    nc = _NC_CACHE[key]
    kw = {}
    if _trace:
        kw = dict(trace=True, **(_trace_kwargs or {}))
    res = run_bass_kernel_spmd(nc, in_maps, list(range(NCORES)), **kw)
    outs = [np.asarray(res.results[c]["out"]) for c in range(NCORES)]
    full = np.concatenate(outs, axis=0)  # [1024, 128, 512] fp16, (h,j) packed
    out = full.astype(np.float32).reshape(nb * nloc, nnei, NH, nnei) \
        .transpose(0, 1, 3, 2).reshape(nb, nloc, nnei, nnei, NH)
    out = np.ascontiguousarray(out)
    if _trace:
        return out, res
    return out


if __name__ == "__main__":
    import reference as R
    inputs = {k: np.asarray(v) for k, v in R.setup_inputs().items()}
    out = kernel(**inputs)
    import jax.numpy as jnp
    ref = np.asarray(R.reference(**{k: jnp.asarray(v) for k, v in inputs.items()}))
    err = np.abs(out - ref)
    scale = np.abs(ref).max()
    print("absmax err:", err.max(), "scale:", scale, "scale-rel:", err.max() / scale)
    print("rel L2:", np.linalg.norm(err) / np.linalg.norm(ref))
